# revision 1
# baseline (speedup 1.0000x reference)
"""Distributed causal multi-head attention for 8 TRN2 NeuronCores.

Problem: x[4,2048,512], 8 heads, causal. out = Attn(x) @ Wo.T + bo.

Sharding: 2 cores per batch element. Each core computes 1024 query rows of
its batch, as four 256-row panels at positions p=0..3 with k-extent
512*(p+1). Core (b,0) takes rows [512p, 512p+256), core (b,1) takes
[512p+256, 512p+512); both run the identical SPMD graph - all per-core
differences flow through input data (gathered query rows, mask tiles).

Device layouts are transposed so that per-query softmax reductions become
matmuls (ones-vector contraction) instead of partition reductions:
  QT[j,q], KT[j,k] from  W.T @ x.T ;  V[k,j] natural;
  S^T[k,q] = KT_head.T @ QT_head (two heads row-packed 0-63 / 64-127);
  P = exp(S^T/8 + mask) on ScalarE; o^T[d,q] accumulated over k-blocks
  with the two heads col-packed; den via ones[128,1] matmul;
  out[q,:] = attT.T @ Wo.T + bo with attT as lhsT directly.
"""

import os
import sys

import numpy as np

sys.path.insert(0, "/opt/trn_rl_repo")

import concourse.bass as bass  # noqa: E402
import concourse.mybir as mybir  # noqa: E402
from concourse import bacc  # noqa: E402
from concourse.tile import TileContext  # noqa: E402

P = 128
D = 512
S = 2048
H = 8
DH = 64
NPANEL = 4
QP = 256  # query rows per panel
NQ = NPANEL * QP  # 1024 query rows per core
NEG = -1.0e9
SCALE = 0.125  # 1/sqrt(DH)

MMDT_NAME = os.environ.get("KERNEL_MMDT", "bf16")  # bf16 | f32r | f32

f32 = mybir.dt.float32
Exp = mybir.ActivationFunctionType.Exp
add_op = mybir.AluOpType.add
mult_op = mybir.AluOpType.mult



# Matmul-operand dtype. bf16 runs 4x faster than f32 on PE (1 cyc/row) and
# halves SBUF/DMA for the big tensors; PSUM accumulation stays f32.
# (float32r is also 1 cyc/row but its self-loading weight path in walrus
# supports only one sync-wait slot -> "Too many sync wait commands".)
MMDT = {"bf16": mybir.dt.bfloat16, "f32r": mybir.dt.float32r, "f32": f32}[MMDT_NAME]


def _mm(ap):
    return ap


def build():
    # Bacc (not Bass): its compile() pipeline runs generate_event_semaphores,
    # which splits multi-wait instructions to satisfy the 1-wait-per-
    # instruction hardware limit.
    nc = bacc.Bacc()

    xT = nc.declare_dram_parameter("xT", [D, S], MMDT, isOutput=False)
    xqT = nc.declare_dram_parameter("xqT", [D, NQ], MMDT, isOutput=False)
    wqT = nc.declare_dram_parameter("wqT", [D, D], MMDT, isOutput=False)
    wkT = nc.declare_dram_parameter("wkT", [D, D], MMDT, isOutput=False)
    wvT = nc.declare_dram_parameter("wvT", [D, D], MMDT, isOutput=False)
    woT = nc.declare_dram_parameter("woT", [DH, H, D], MMDT, isOutput=False)
    bq = nc.declare_dram_parameter("bq", [P, 4], f32, isOutput=False)
    bk = nc.declare_dram_parameter("bk", [P, 4], f32, isOutput=False)
    bv_bc = nc.declare_dram_parameter("bv_bc", [P, D], f32, isOutput=False)
    bo_bc = nc.declare_dram_parameter("bo_bc", [P, D], f32, isOutput=False)
    mask = nc.declare_dram_parameter("mask", [P, NPANEL, 4, QP], MMDT, isOutput=False)
    ones128 = nc.declare_dram_parameter("ones128", [P, 1], MMDT, isOutput=False)
    ones64 = nc.declare_dram_parameter("ones64", [1, DH], MMDT, isOutput=False)
    out = nc.declare_dram_parameter("out", [NQ, D], f32, isOutput=True)

    with nc.allow_low_precision(reason="float32r matmul operands (rounded fp32)"), TileContext(nc) as tc:
        with (
            tc.tile_pool(name="big", bufs=1) as bpool,
            tc.tile_pool(name="attp", bufs=2) as apool,
            tc.tile_pool(name="work", bufs=4) as wpool,
            tc.tile_pool(name="osb", bufs=2) as opool,
            tc.tile_pool(name="ps_proj", bufs=2, space="PSUM") as ps_proj,
            tc.tile_pool(name="ps_s", bufs=2, space="PSUM") as ps_s,
            tc.tile_pool(name="ps_ot", bufs=2, space="PSUM") as ps_ot,
        ):
            # ---- persistent SBUF tensors ----
            xT_sb = bpool.tile([P, 4, S], MMDT, tag="xT")
            xqT_sb = bpool.tile([P, 4, NQ], MMDT, tag="xqT")
            # K^T/Q^T stored head-major on partitions 0-63: matmuls with
            # operands at SBUF base-partition 64 (auto row tile_position)
            # are device-fatal (NRT_EXEC_UNIT_UNRECOVERABLE), so every
            # attention matmul must run at partition base 0.
            kT_sb = bpool.tile([DH, H, S], MMDT, tag="kT")
            v_sb = bpool.tile([P, S // P, H, DH + 1], MMDT, tag="v")
            qT_sb = bpool.tile([DH, H, NQ], MMDT, tag="qT")
            w_sb = {}
            for name, prm in (("wq", wqT), ("wk", wkT), ("wv", wvT)):
                w_sb[name] = bpool.tile([P, 4, D], MMDT, tag=name, name=name)
                nc.sync.dma_start(
                    out=w_sb[name][:], in_=prm[:].rearrange("(o p) j -> p o j", p=P)
                )
            # O-proj weights head-major at partition base 0: [dh, head, n]
            wo_sb = bpool.tile([DH, H, D], MMDT, tag="wo")
            nc.sync.dma_start(out=wo_sb[:], in_=woT[:])
            bq_sb = bpool.tile([P, 4], f32, tag="bq")
            bk_sb = bpool.tile([P, 4], f32, tag="bk")
            bv_sb = bpool.tile([P, D], f32, tag="bv")
            bo_sb = bpool.tile([P, D], f32, tag="bo")
            mask_sb = bpool.tile([P, NPANEL, 4, QP], MMDT, tag="mask")
            o128_sb = bpool.tile([P, 1], MMDT, tag="o128")
            o64_sb = bpool.tile([1, DH], MMDT, tag="o64")

            nc.sync.dma_start(out=xT_sb[:], in_=xT[:].rearrange("(o p) q -> p o q", p=P))
            nc.sync.dma_start(out=xqT_sb[:], in_=xqT[:].rearrange("(o p) q -> p o q", p=P))
            nc.sync.dma_start(out=bq_sb[:], in_=bq[:])
            nc.sync.dma_start(out=bk_sb[:], in_=bk[:])
            nc.sync.dma_start(out=bv_sb[:], in_=bv_bc[:])
            nc.sync.dma_start(out=bo_sb[:], in_=bo_bc[:])
            nc.sync.dma_start(out=mask_sb[:], in_=mask[:])
            nc.sync.dma_start(out=o128_sb[:], in_=ones128[:])
            nc.sync.dma_start(out=o64_sb[:], in_=ones64[:])

            # ---- projections ----
            # KT[j,k] = sum_d Wk.T[d,j] * xT[d,k]  (+bk per-partition j)
            for jb in range(4):
                for kc in range(4):
                    ps = ps_proj.tile([P, 512], f32, tag="p512")
                    for db in range(4):
                        nc.tensor.matmul(
                            ps[:],
                            lhsT=_mm(w_sb["wk"][:, db, jb * P : (jb + 1) * P]),
                            rhs=_mm(xT_sb[:, db, kc * 512 : (kc + 1) * 512]),
                            start=(db == 0),
                            stop=(db == 3),
                        )
                    stg = wpool.tile([P, 512], MMDT, tag="stage")
                    nc.vector.tensor_tensor(
                        stg[:],
                        ps[:],
                        bk_sb[:, jb : jb + 1].to_broadcast([P, 512]),
                        add_op,
                    )
                    for hh in range(2):
                        nc.sync.dma_start(
                            out=kT_sb[:, 2 * jb + hh, kc * 512 : (kc + 1) * 512],
                            in_=stg[hh * DH : (hh + 1) * DH, :],
                        )
            # QT[j,q] likewise from xqT (q = this core's 1024 rows)
            for jb in range(4):
                for qc in range(2):
                    ps = ps_proj.tile([P, 512], f32, tag="p512")
                    for db in range(4):
                        nc.tensor.matmul(
                            ps[:],
                            lhsT=_mm(w_sb["wq"][:, db, jb * P : (jb + 1) * P]),
                            rhs=_mm(xqT_sb[:, db, qc * 512 : (qc + 1) * 512]),
                            start=(db == 0),
                            stop=(db == 3),
                        )
                    stg = wpool.tile([P, 512], MMDT, tag="stage")
                    nc.vector.tensor_tensor(
                        stg[:],
                        ps[:],
                        bq_sb[:, jb : jb + 1].to_broadcast([P, 512]),
                        add_op,
                    )
                    for hh in range(2):
                        nc.sync.dma_start(
                            out=qT_sb[:, 2 * jb + hh, qc * 512 : (qc + 1) * 512],
                            in_=stg[hh * DH : (hh + 1) * DH, :],
                        )
            # V[k,j] = sum_d xT[d,k] * Wv.T[d,j]  (+bv per free-dim j), with a
            # ones column appended per head so P.V also yields the softmax
            # denominator in psum row DH for free
            nc.vector.memset(v_sb[:, :, :, DH : DH + 1], 1.0)
            for kb in range(S // P):
                ps = ps_proj.tile([P, 512], f32, tag="p512")
                for db in range(4):
                    nc.tensor.matmul(
                        ps[:],
                        lhsT=_mm(xT_sb[:, db, kb * P : (kb + 1) * P]),
                        rhs=_mm(w_sb["wv"][:, db, :]),
                        start=(db == 0),
                        stop=(db == 3),
                    )
                nc.vector.tensor_tensor(
                    v_sb[:, kb, :, 0:DH],
                    ps[:].rearrange("p (h d) -> p h d", h=H),
                    bv_sb[:].rearrange("p (h d) -> p h d", h=H),
                    add_op,
                )

            # ---- attention ----
            for p in range(NPANEL):
                nblk = 4 * (p + 1)
                nbat = nblk // 2  # 2 k-blocks per exp batch
                q0 = p * QP
                for hp in range(4):  # head pairs (2hp, 2hp+1)
                    # heads stack along the free dim (same partitions 0-63)
                    # so PSUM accumulation groups never mix partition bases
                    ot_ps = ps_ot.tile([DH + 1, 2, QP], f32, tag="ot")

                    def emit_pv(bb, pT, hp=hp, nbat=nbat, ot_ps=ot_ps):
                        for kbi in range(2):
                            for hh in range(2):
                                h = 2 * hp + hh
                                nc.tensor.matmul(
                                    ot_ps[:, hh, :],
                                    lhsT=_mm(v_sb[:, 2 * bb + kbi, h, :]),
                                    rhs=_mm(pT[:, kbi, hh, :]),
                                    start=(bb == 0 and kbi == 0 and hh == 0),
                                    stop=(
                                        bb == nbat - 1 and kbi == 1 and hh == 1
                                    ),
                                )

                    # software pipeline: PV for batch bb-1 is emitted after
                    # scores+exp of batch bb, so the in-order PE queue always
                    # has ready matmuls while ACT runs the exp
                    pending = []
                    for bb in range(nbat):
                        s_ps = ps_s.tile([P, 2, 2, QP], f32, tag="s")
                        for kbi in range(2):
                            kb = 2 * bb + kbi
                            for hh in range(2):
                                h = 2 * hp + hh
                                nc.tensor.matmul(
                                    s_ps[:, kbi, hh, :],
                                    lhsT=_mm(kT_sb[:, h, kb * P : (kb + 1) * P]),
                                    rhs=_mm(qT_sb[:, h, q0 : q0 + QP]),
                                    start=True,
                                    stop=True,
                                )
                        pT = wpool.tile([P, 2, 2, QP], MMDT, tag="pT")
                        nc.scalar.activation(pT[:], s_ps[:], Exp, scale=SCALE)
                        for kbi in range(2):
                            kb = 2 * bb + kbi
                            if kb >= nblk - 4:
                                # zero masked probabilities: bf16 SBUF in/out
                                # hits the DVE 4x mode
                                i = kb - (nblk - 4)
                                nc.vector.tensor_tensor(
                                    pT[:, kbi, :, :],
                                    pT[:, kbi, :, :],
                                    mask_sb[:, p, i : i + 1, :].to_broadcast(
                                        [P, 2, QP]
                                    ),
                                    mult_op,
                                )
                        pending.append((bb, pT))
                        if len(pending) > 2:
                            emit_pv(*pending.pop(0))
                    for item in pending:
                        emit_pv(*item)
                    # normalize: attT[:,hp,:] = ot / den  (per-head bcast of 1/den)
                    den_sb = wpool.tile([1, 2, QP], f32, tag="den_sb")
                    nc.vector.tensor_copy(out=den_sb[:], in_=ot_ps[DH : DH + 1, :, :])
                    rden_f = wpool.tile([1, 2, QP], f32, tag="rden_f")
                    nc.vector.reciprocal_approx_fast(out=rden_f[:], in_=den_sb[:])
                    rden = wpool.tile([1, 2, QP], MMDT, tag="rden")
                    nc.vector.tensor_copy(out=rden[:], in_=rden_f[:])
                    bc_full = ps_s.tile([P, 2, 2, QP], f32, tag="s", name="bc_full")
                    bc_ps = bc_full[0:DH, 0, :, :]
                    nc.tensor.matmul(
                        bc_ps, lhsT=_mm(o64_sb[:]), rhs=_mm(rden[:]),
                        start=True, stop=True,
                    )
                    bc_sb = wpool.tile([DH, 2, QP], f32, tag="bc_sb")
                    nc.vector.tensor_copy(out=bc_sb[:], in_=bc_ps)
                    if hp == 0:
                        attT_sb = apool.tile([DH, H, QP], MMDT, tag="attT")
                    nc.vector.tensor_mul(
                        out=attT_sb[:, 2 * hp : 2 * hp + 2, :],
                        in0=ot_ps[0:DH, :, :],
                        in1=bc_sb[:],
                    )
                # O projection for this panel's 256 rows (two 128-row subtiles)
                for qs in range(2):
                    ps = ps_proj.tile([P, D], f32, tag="p512")
                    for hb in range(H):
                        nc.tensor.matmul(
                            ps[:],
                            lhsT=_mm(attT_sb[:, hb, qs * P : (qs + 1) * P]),
                            rhs=_mm(wo_sb[:, hb, :]),
                            start=(hb == 0),
                            stop=(hb == H - 1),
                        )
                    osb = opool.tile([P, D], f32, tag="osb")
                    nc.vector.tensor_tensor(osb[:], ps[:], bo_sb[:], add_op)
                    nc.sync.dma_start(
                        out=out[p * QP + qs * P : p * QP + (qs + 1) * P, :], in_=osb[:]
                    )
    return nc


_NC = None


def _get_nc():
    global _NC
    if _NC is None:
        _NC = build()
        # run_bass_via_pjrt does not finalize; Bacc.finalize runs the compile
        # passes (register allocation, event-semaphore wait splitting).
        _NC.finalize()
    return _NC


def _qrows(half):
    return np.concatenate(
        [np.arange(512 * p + 256 * half, 512 * p + 256 * half + QP) for p in range(NPANEL)]
    )


def _mask_for(half, mmnp):
    m = np.empty((P, NPANEL, 4, QP), np.float32)
    r = np.arange(P)[:, None]
    c = np.arange(QP)[None, :]
    for p in range(NPANEL):
        q0 = 512 * p + 256 * half
        for i in range(4):
            k = (4 * p + i) * P + r
            m[:, p, i, :] = np.where(k <= q0 + c, 1.0, 0.0)
    return m.astype(mmnp)


def _in_maps(inputs):
    mmnp = mybir.dt.np(MMDT)
    x = np.asarray(inputs["x"], np.float32)
    wq = np.ascontiguousarray(np.asarray(inputs["W_Q_w"], np.float32).T).astype(mmnp)
    wk = np.ascontiguousarray(np.asarray(inputs["W_K_w"], np.float32).T).astype(mmnp)
    wv = np.ascontiguousarray(np.asarray(inputs["W_V_w"], np.float32).T).astype(mmnp)
    wo = np.ascontiguousarray(
        np.asarray(inputs["W_O_w"], np.float32).T.reshape(H, DH, D).transpose(1, 0, 2)
    ).astype(mmnp)
    bq = np.ascontiguousarray(np.asarray(inputs["W_Q_b"], np.float32).reshape(4, P).T)
    bk = np.ascontiguousarray(np.asarray(inputs["W_K_b"], np.float32).reshape(4, P).T)
    bv_bc = np.ascontiguousarray(
        np.broadcast_to(np.asarray(inputs["W_V_b"], np.float32), (P, D))
    )
    bo_bc = np.ascontiguousarray(
        np.broadcast_to(np.asarray(inputs["W_O_b"], np.float32), (P, D))
    )
    ones128 = np.ones((P, 1), mmnp)
    ones64 = np.ones((1, DH), mmnp)
    masks = [_mask_for(0, mmnp), _mask_for(1, mmnp)]
    in_maps = []
    for core in range(8):
        b, half = core // 2, core % 2
        xb = x[b]
        in_maps.append(
            {
                "xT": np.ascontiguousarray(xb.T).astype(mmnp),
                "xqT": np.ascontiguousarray(xb[_qrows(half)].T).astype(mmnp),
                "wqT": wq,
                "wkT": wk,
                "wvT": wv,
                "woT": wo,
                "bq": bq,
                "bk": bk,
                "bv_bc": bv_bc,
                "bo_bc": bo_bc,
                "mask": masks[half],
                "ones128": ones128,
                "ones64": ones64,
            }
        )
    return in_maps


def _assemble(results, B=4):
    out = np.empty((B, S, D), np.float32)
    for core in range(8):
        b, half = core // 2, core % 2
        out[b, _qrows(half), :] = results[core]["out"]
    return out


def run(inputs, trace=False, **kw):
    from concourse.bass_utils import run_bass_kernel_spmd

    res = run_bass_kernel_spmd(
        _get_nc(), _in_maps(inputs), core_ids=list(range(8)), trace=trace, **kw
    )
    return _assemble(res.results), res


def kernel(**inputs):
    out, _ = run(inputs, trace=False)
    return out



# revision 5
# speedup vs baseline: 1.5440x; 1.5440x over previous
"""Distributed causal multi-head attention for 8 TRN2 NeuronCores.

Problem: x[4,2048,512], 8 heads, causal. out = Attn(x) @ Wo.T + bo.

Sharding: 2 cores per batch element. Each core computes 1024 query rows of
its batch, as four 256-row panels at positions p=0..3 with k-extent
512*(p+1). Core (b,0) takes rows [512p, 512p+256), core (b,1) takes
[512p+256, 512p+512); both run the identical SPMD graph - all per-core
differences flow through input data (gathered query rows, mask tiles).

Device layouts are transposed so that per-query softmax reductions become
matmuls (ones-vector contraction) instead of partition reductions:
  QT[j,q], KT[j,k] from  W.T @ x.T ;  V[k,j] natural;
  S^T[k,q] = KT_head.T @ QT_head (two heads row-packed 0-63 / 64-127);
  P = exp(S^T/8) * mask on ScalarE/DVE; o^T[d,q] accumulated over k-blocks
  with the two heads col-packed; den via ones-column of V.

v2 structure (vs the first working version):
  - chunk-pipelined: projections for k-chunk kc are emitted right before
    attention panel kc (panel p only needs k < 512(p+1)), so ScalarE exp
    of panel kc overlaps PE projections of chunk kc+1.
  - K bias dropped (adds a per-q constant to scores -> cancels in softmax);
    V bias folded host-side into bo' = bo + bv @ Wo.T.
  - head-split of K/Q projections via DVE copies out of PSUM instead of
    SBUF->SBUF DMA staging.
  - mask pre-expanded over the head dim so the DVE multiply has no
    broadcast operand (2x perf mode).
  - the normalize chain (recip/cast/ones-broadcast-matmul/muls) of head
    pair hp is emitted *inside* head pair hp+1's score stream, so the
    in-order PE queue never head-of-line blocks on the DVE chain (this
    stall previously re-throttled HAM to 1.2 GHz kernel-wide).
  - O-projection packs head pairs on 128 partitions (K=128 contraction)
    and is deferred behind the next panel's scores.
"""

import os
import sys

import numpy as np

sys.path.insert(0, "/opt/trn_rl_repo")

import concourse.bass as bass  # noqa: E402
import concourse.mybir as mybir  # noqa: E402
from concourse import bacc  # noqa: E402
from concourse.tile import TileContext  # noqa: E402

P = 128
D = 512
S = 2048
H = 8
DH = 64
NPANEL = 4
QP = 256  # query rows per panel
NQ = NPANEL * QP  # 1024 query rows per core
SCALE = 0.125  # 1/sqrt(DH)

MMDT_NAME = os.environ.get("KERNEL_MMDT", "bf16")

f32 = mybir.dt.float32
Exp = mybir.ActivationFunctionType.Exp
add_op = mybir.AluOpType.add
mult_op = mybir.AluOpType.mult

MMDT = {"bf16": mybir.dt.bfloat16, "f32r": mybir.dt.float32r, "f32": f32}[MMDT_NAME]


def build():
    # Bacc (not Bass): its compile() pipeline runs generate_event_semaphores,
    # which splits multi-wait instructions to satisfy the 1-wait-per-
    # instruction hardware limit.
    nc = bacc.Bacc()

    xT = nc.declare_dram_parameter("xT", [D, S], MMDT, isOutput=False)
    xqT = nc.declare_dram_parameter("xqT", [D, NQ], MMDT, isOutput=False)
    wqT = nc.declare_dram_parameter("wqT", [D, D], MMDT, isOutput=False)
    wkT = nc.declare_dram_parameter("wkT", [D, D], MMDT, isOutput=False)
    wvT = nc.declare_dram_parameter("wvT", [D, D], MMDT, isOutput=False)
    woT2 = nc.declare_dram_parameter("woT2", [P, 4, D], MMDT, isOutput=False)
    bq = nc.declare_dram_parameter("bq", [P, 4], f32, isOutput=False)
    bo_bc = nc.declare_dram_parameter("bo_bc", [P, D], f32, isOutput=False)
    mask = nc.declare_dram_parameter("mask", [P, NPANEL, 4, 2, QP], MMDT, isOutput=False)
    ones64 = nc.declare_dram_parameter("ones64", [1, DH], MMDT, isOutput=False)
    out = nc.declare_dram_parameter("out", [NQ, D], f32, isOutput=True)

    with nc.allow_low_precision(reason="bf16 matmul operands"), TileContext(nc) as tc:
        with (
            tc.tile_pool(name="big", bufs=1) as bpool,
            tc.tile_pool(name="attp", bufs=2) as apool,
            tc.tile_pool(name="work", bufs=4) as wpool,
            tc.tile_pool(name="osb", bufs=2) as opool,
            tc.tile_pool(name="ps_proj", bufs=2, space="PSUM") as ps_proj,
            tc.tile_pool(name="ps_s", bufs=2, space="PSUM") as ps_s,
            tc.tile_pool(name="ps_ot", bufs=2, space="PSUM") as ps_ot,
        ):
            # ---- persistent SBUF tensors ----
            # x chunk-major: [p, kc, db, 512] so each k-chunk DMA is contiguous
            xT_sb = bpool.tile([P, 4, 4, 512], MMDT, tag="xT")
            xqT_sb = bpool.tile([P, NPANEL, 4, QP], MMDT, tag="xqT")
            # K^T/Q^T stored head-major on partitions 0-63: matmuls with
            # operands at SBUF base-partition 64 (auto row tile_position)
            # are device-fatal (NRT_EXEC_UNIT_UNRECOVERABLE), so every
            # attention matmul must run at partition base 0.
            kT_sb = bpool.tile([DH, H, S], MMDT, tag="kT")
            v_sb = bpool.tile([P, S // P, H, DH + 1], MMDT, tag="v")
            qT_sb = bpool.tile([DH, H, NQ], MMDT, tag="qT")
            w_sb = {}
            for name, prm in (("wk", wkT), ("wq", wqT), ("wv", wvT)):
                w_sb[name] = bpool.tile([P, 4, D], MMDT, tag=name, name=name)
            wo_sb = bpool.tile([P, 4, D], MMDT, tag="wo")
            bq_sb = bpool.tile([P, 4], f32, tag="bq")
            bo_sb = bpool.tile([P, D], f32, tag="bo")
            mask_sb = bpool.tile([P, NPANEL, 4, 2, QP], MMDT, tag="mask")
            o64_sb = bpool.tile([1, DH], MMDT, tag="o64")

            def dma_chunk(kc):
                nc.sync.dma_start(
                    out=xT_sb[:, kc],
                    in_=xT[:, kc * 512 : (kc + 1) * 512].rearrange(
                        "(o p) q -> p o q", p=P
                    ),
                )
                nc.sync.dma_start(
                    out=xqT_sb[:, kc],
                    in_=xqT[:, kc * QP : (kc + 1) * QP].rearrange(
                        "(o p) q -> p o q", p=P
                    ),
                )
                nc.sync.dma_start(out=mask_sb[:, kc], in_=mask[:, kc])

            # input DMAs emitted in the order compute consumes them: the first
            # K-projection needs only wk + x chunk 0; Wo/bo are first read at
            # panel 0's O-projection, ~30us in
            for name, prm in (("wk", wkT), ("wq", wqT), ("wv", wvT)):
                nc.sync.dma_start(
                    out=w_sb[name][:], in_=prm[:].rearrange("(o p) j -> p o j", p=P)
                )
            nc.sync.dma_start(out=bq_sb[:], in_=bq[:])
            dma_chunk(0)
            nc.sync.dma_start(out=o64_sb[:], in_=ones64[:])
            dma_chunk(1)
            nc.sync.dma_start(out=wo_sb[:], in_=woT2[:])
            nc.sync.dma_start(out=bo_sb[:], in_=bo_bc[:])
            dma_chunk(2)
            dma_chunk(3)
            # ones column appended per head so P.V also yields the softmax
            # denominator in psum row DH for free
            nc.vector.memset(v_sb[:, :, :, DH : DH + 1], 1.0)

            # deferred emission slots (see module docstring)
            deferred = {"norm": None, "oproj": None}

            def emit_proj_chunk(kc):
                # K^T[j, k-chunk] = sum_d Wk.T[d,j] xT[d,k]; no bias (cancels
                # in softmax); split to head-major planes via DVE copies
                for jb in range(4):
                    ps = ps_proj.tile([P, 512], f32, tag="p512")
                    for db in range(4):
                        nc.tensor.matmul(
                            ps[:],
                            lhsT=w_sb["wk"][:, db, jb * P : (jb + 1) * P],
                            rhs=xT_sb[:, kc, db, :],
                            start=(db == 0),
                            stop=(db == 3),
                        )
                    for hh in range(2):
                        nc.vector.tensor_copy(
                            out=kT_sb[:, 2 * jb + hh, kc * 512 : (kc + 1) * 512],
                            in_=ps[hh * DH : (hh + 1) * DH, :],
                        )
                # Q^T[j, q-panel kc] with bias bq
                for jb in range(4):
                    ps = ps_proj.tile([P, 512], f32, tag="p512")
                    psq = ps[:, 0:QP]
                    for db in range(4):
                        nc.tensor.matmul(
                            psq,
                            lhsT=w_sb["wq"][:, db, jb * P : (jb + 1) * P],
                            rhs=xqT_sb[:, kc, db, :],
                            start=(db == 0),
                            stop=(db == 3),
                        )
                    for hh in range(2):
                        nc.vector.tensor_tensor(
                            qT_sb[:, 2 * jb + hh, kc * QP : (kc + 1) * QP],
                            ps[hh * DH : (hh + 1) * DH, 0:QP],
                            bq_sb[hh * DH : (hh + 1) * DH, jb : jb + 1].to_broadcast(
                                [DH, QP]
                            ),
                            add_op,
                        )
                # V[k-chunk, j] = sum_d xT[d,k] Wv.T[d,j]; no bias (folded
                # into bo' host-side)
                for kb in range(4):
                    ps = ps_proj.tile([P, 512], f32, tag="p512")
                    for db in range(4):
                        nc.tensor.matmul(
                            ps[:],
                            lhsT=xT_sb[:, kc, db, kb * P : (kb + 1) * P],
                            rhs=w_sb["wv"][:, db, :],
                            start=(db == 0),
                            stop=(db == 3),
                        )
                    nc.vector.tensor_copy(
                        out=v_sb[:, 4 * kc + kb, :, 0:DH],
                        in_=ps[:].rearrange("p (h d) -> p h d", h=H),
                    )

            def make_norm(p, hp, ot_ps, attT_sb):
                def emit_norm():
                    # attT[:, hp pair, :] = ot / den; den sits in psum row DH.
                    # reciprocal reads PSUM directly; the ones-matmul
                    # broadcasts 1/den across 64 partitions on PE (emitted
                    # here, behind independent score matmuls, so PE never
                    # stalls on the DVE chain).
                    # custom-DVE recip must read SBUF (PSUM input returns
                    # garbage on HW even though CoreSim accepts it)
                    den_sb = wpool.tile([1, 2, QP], f32, tag="den_sb")
                    nc.vector.tensor_copy(out=den_sb[:], in_=ot_ps[DH : DH + 1, :, :])
                    rden_f = wpool.tile([1, 2, QP], f32, tag="rden_f")
                    nc.vector.reciprocal_approx_fast(out=rden_f[:], in_=den_sb[:])
                    rden = wpool.tile([1, 2, QP], MMDT, tag="rden")
                    nc.vector.tensor_copy(out=rden[:], in_=rden_f[:])
                    bc_full = ps_s.tile([P, 2, 2, QP], f32, tag="s", name="bc")
                    bc_ps = bc_full[0:DH, 0, :, :]
                    nc.tensor.matmul(
                        bc_ps, lhsT=o64_sb[:], rhs=rden[:], start=True, stop=True
                    )
                    # DVE cannot read two PSUM operands in one op (NCC_IBVF027)
                    # so stage the broadcast through SBUF
                    bc_sb = wpool.tile([DH, 2, QP], f32, tag="bc_sb")
                    nc.vector.tensor_copy(out=bc_sb[:], in_=bc_ps)
                    for hh in range(2):
                        nc.vector.tensor_mul(
                            out=attT_sb[hh * DH : (hh + 1) * DH, hp, :],
                            in0=ot_ps[0:DH, hh, :],
                            in1=bc_sb[:, hh, :],
                        )

                return emit_norm

            def make_oproj(p, attT_sb):
                def emit_oproj():
                    # out[q,:] = attT.T @ Wo.T + bo', head pairs packed so the
                    # contraction uses all 128 partitions
                    for qs in range(2):
                        ps = ps_proj.tile([P, D], f32, tag="p512")
                        for hp in range(4):
                            nc.tensor.matmul(
                                ps[:],
                                lhsT=attT_sb[:, hp, qs * P : (qs + 1) * P],
                                rhs=wo_sb[:, hp, :],
                                start=(hp == 0),
                                stop=(hp == 3),
                            )
                        osb = opool.tile([P, D], f32, tag="osb")
                        nc.vector.tensor_tensor(osb[:], ps[:], bo_sb[:], add_op)
                        nc.sync.dma_start(
                            out=out[p * QP + qs * P : p * QP + (qs + 1) * P, :],
                            in_=osb[:],
                        )

                return emit_oproj

            def emit_attention_panel(p):
                nblk = 4 * (p + 1)
                nbat = nblk // 2  # 2 k-blocks per exp batch
                q0 = p * QP
                norm_at = 1 if nbat == 2 else 2
                attT_sb = apool.tile([P, 4, QP], MMDT, tag="attT")
                for hp in range(4):  # head pairs (2hp, 2hp+1)
                    ot_ps = ps_ot.tile([DH + 1, 2, QP], f32, tag="ot")

                    def emit_pv(bb, pT, hp=hp, nbat=nbat, ot_ps=ot_ps):
                        for kbi in range(2):
                            for hh in range(2):
                                h = 2 * hp + hh
                                nc.tensor.matmul(
                                    ot_ps[:, hh, :],
                                    lhsT=v_sb[:, 2 * bb + kbi, h, :],
                                    rhs=pT[:, kbi, hh, :],
                                    start=(bb == 0 and kbi == 0 and hh == 0),
                                    stop=(bb == nbat - 1 and kbi == 1 and hh == 1),
                                )

                    # software pipeline: PV for batch bb-1 is emitted after
                    # scores+exp of batch bb, so the in-order PE queue always
                    # has ready matmuls while ACT runs the exp
                    pending = []
                    for bb in range(nbat):
                        s_ps = ps_s.tile([P, 2, 2, QP], f32, tag="s")
                        for kbi in range(2):
                            kb = 2 * bb + kbi
                            for hh in range(2):
                                h = 2 * hp + hh
                                nc.tensor.matmul(
                                    s_ps[:, kbi, hh, :],
                                    lhsT=kT_sb[:, h, kb * P : (kb + 1) * P],
                                    rhs=qT_sb[:, h, q0 : q0 + QP],
                                    start=True,
                                    stop=True,
                                )
                        pT = wpool.tile([P, 2, 2, QP], MMDT, tag="pT")
                        nc.scalar.activation(pT[:], s_ps[:], Exp, scale=SCALE)
                        for kbi in range(2):
                            kb = 2 * bb + kbi
                            if kb >= nblk - 4:
                                # zero masked probabilities; mask pre-expanded
                                # over the head dim -> no broadcast operand ->
                                # DVE 2x mode
                                i = kb - (nblk - 4)
                                nc.vector.tensor_tensor(
                                    pT[:, kbi, :, :],
                                    pT[:, kbi, :, :],
                                    mask_sb[:, p, i, :, :],
                                    mult_op,
                                )
                        pending.append((bb, pT))
                        if bb == norm_at and deferred["norm"] is not None:
                            deferred["norm"]()
                            deferred["norm"] = None
                        if (
                            hp == 0
                            and bb == nbat - 1
                            and deferred["oproj"] is not None
                        ):
                            deferred["oproj"]()
                            deferred["oproj"] = None
                        if len(pending) > 2:
                            emit_pv(*pending.pop(0))
                    for item in pending:
                        emit_pv(*item)
                    deferred["norm"] = make_norm(p, hp, ot_ps, attT_sb)
                deferred["oproj"] = make_oproj(p, attT_sb)

            for kc in range(4):
                emit_proj_chunk(kc)
                emit_attention_panel(kc)
            deferred["norm"]()
            deferred["oproj"]()
    return nc


_NC = None


def _get_nc():
    global _NC
    if _NC is None:
        _NC = build()
        # run_bass_via_pjrt does not finalize; Bacc.finalize runs the compile
        # passes (register allocation, event-semaphore wait splitting).
        _NC.finalize()
    return _NC


def _qrows(half):
    return np.concatenate(
        [np.arange(512 * p + 256 * half, 512 * p + 256 * half + QP) for p in range(NPANEL)]
    )


def _mask_for(half, mmnp):
    m = np.empty((P, NPANEL, 4, 2, QP), np.float32)
    r = np.arange(P)[:, None]
    c = np.arange(QP)[None, :]
    for p in range(NPANEL):
        q0 = 512 * p + 256 * half
        for i in range(4):
            k = (4 * p + i) * P + r
            mi = np.where(k <= q0 + c, 1.0, 0.0)
            m[:, p, i, 0, :] = mi
            m[:, p, i, 1, :] = mi
    return m.astype(mmnp)


def _in_maps(inputs):
    mmnp = mybir.dt.np(MMDT)
    x = np.asarray(inputs["x"], np.float32)
    wq = np.ascontiguousarray(np.asarray(inputs["W_Q_w"], np.float32).T).astype(mmnp)
    wk = np.ascontiguousarray(np.asarray(inputs["W_K_w"], np.float32).T).astype(mmnp)
    wv = np.ascontiguousarray(np.asarray(inputs["W_V_w"], np.float32).T).astype(mmnp)
    woT = np.asarray(inputs["W_O_w"], np.float32).T  # [ (h,dh), n ]
    wo2 = np.ascontiguousarray(
        woT.reshape(4, P, D).transpose(1, 0, 2)
    ).astype(mmnp)
    bq = np.ascontiguousarray(np.asarray(inputs["W_Q_b"], np.float32).reshape(4, P).T)
    # V bias folded through the O projection: bo' = bo + bv @ Wo.T
    bo_eff = np.asarray(inputs["W_O_b"], np.float32) + (
        np.asarray(inputs["W_V_b"], np.float32) @ woT
    )
    bo_bc = np.ascontiguousarray(np.broadcast_to(bo_eff, (P, D)))
    ones64 = np.ones((1, DH), mmnp)
    masks = [_mask_for(0, mmnp), _mask_for(1, mmnp)]
    in_maps = []
    for core in range(8):
        b, half = core // 2, core % 2
        xb = x[b]
        in_maps.append(
            {
                "xT": np.ascontiguousarray(xb.T).astype(mmnp),
                "xqT": np.ascontiguousarray(xb[_qrows(half)].T).astype(mmnp),
                "wqT": wq,
                "wkT": wk,
                "wvT": wv,
                "woT2": wo2,
                "bq": bq,
                "bo_bc": bo_bc,
                "mask": masks[half],
                "ones64": ones64,
            }
        )
    return in_maps


def _assemble(results, B=4):
    out = np.empty((B, S, D), np.float32)
    for core in range(8):
        b, half = core // 2, core % 2
        out[b, _qrows(half), :] = results[core]["out"]
    return out


def run(inputs, trace=False, **kw):
    from concourse.bass_utils import run_bass_kernel_spmd

    res = run_bass_kernel_spmd(
        _get_nc(), _in_maps(inputs), core_ids=list(range(8)), trace=trace, **kw
    )
    return _assemble(res.results), res


def kernel(**inputs):
    out, _ = run(inputs, trace=False)
    return out


# revision 7
# speedup vs baseline: 1.5968x; 1.0342x over previous
"""Distributed causal multi-head attention for 8 TRN2 NeuronCores.

Problem: x[4,2048,512], 8 heads, causal. out = Attn(x) @ Wo.T + bo.

Sharding: 2 cores per batch element. Each core computes 1024 query rows of
its batch, as four 256-row panels at positions p=0..3 with k-extent
512*(p+1). Core (b,0) takes rows [512p, 512p+256), core (b,1) takes
[512p+256, 512p+512); both run the identical SPMD graph - all per-core
differences flow through input data (gathered query rows, mask tiles).

Device layouts are transposed so that per-query softmax reductions become
matmuls (ones-vector contraction) instead of partition reductions:
  QT[j,q], KT[j,k] from  W.T @ x.T ;  V[k,j] natural;
  S^T[k,q] = KT_head.T @ QT_head (heads on partitions 0-63);
  P = exp(S^T/8) * mask on ScalarE/DVE; o^T[d,q] accumulated over k-blocks
  with the two heads col-packed; den via ones-column of V.

v3 structure:
  - all DRAM inputs pre-packed host-side so every dma_start is a fully
    contiguous [128, bytes] block (>=2KB per partition line, few
    descriptors -> cheap Sync dispatch, full DMA rate).
  - chunk-pipelined and INTERLEAVED: projection matmul groups for k-chunk
    kc+1 are emitted between attention batches of panel kc, so PE always
    has dense ready work while ScalarE runs exp (keeps HAM at 2.4 GHz).
  - K bias dropped (adds a per-q constant to scores -> cancels in
    softmax); V bias folded host-side into bo' = bo + bv @ Wo.T.
  - head-split of K/Q projections via DVE copies out of PSUM (no
    SBUF->SBUF DMA staging).
  - Q projection at N=512 (two panels per matmul group).
  - mask pre-expanded over the head dim so the DVE multiply has no
    broadcast operand (2x perf mode).
  - the normalize chain (den copy/recip/cast/ones-broadcast-matmul/muls)
    of head pair hp is emitted inside head pair hp+1's score stream, so
    the in-order PE queue never head-of-line blocks on the DVE chain.
  - O-projection packs head pairs on 128 partitions (K=128 contraction)
    and is deferred behind the next panel's scores.
"""

import os
import sys

import numpy as np

sys.path.insert(0, "/opt/trn_rl_repo")

import concourse.bass as bass  # noqa: E402
import concourse.mybir as mybir  # noqa: E402
from concourse import bacc  # noqa: E402
from concourse.tile import TileContext  # noqa: E402

P = 128
D = 512
S = 2048
H = 8
DH = 64
NPANEL = 4
QP = 256  # query rows per panel
NQ = NPANEL * QP  # 1024 query rows per core
SCALE = 0.125  # 1/sqrt(DH)

MMDT_NAME = os.environ.get("KERNEL_MMDT", "bf16")

f32 = mybir.dt.float32
Exp = mybir.ActivationFunctionType.Exp
add_op = mybir.AluOpType.add
mult_op = mybir.AluOpType.mult

MMDT = {"bf16": mybir.dt.bfloat16, "f32r": mybir.dt.float32r, "f32": f32}[MMDT_NAME]


def build():
    # Bacc (not Bass): its compile() pipeline runs generate_event_semaphores,
    # which splits multi-wait instructions to satisfy the 1-wait-per-
    # instruction hardware limit.
    nc = bacc.Bacc()

    xTp = nc.declare_dram_parameter("xTp", [4, P, 4, 512], MMDT, isOutput=False)
    xqTp = nc.declare_dram_parameter("xqTp", [2, P, 4, 512], MMDT, isOutput=False)
    wqp = nc.declare_dram_parameter("wqp", [P, 4, D], MMDT, isOutput=False)
    wkp = nc.declare_dram_parameter("wkp", [P, 4, D], MMDT, isOutput=False)
    wvp = nc.declare_dram_parameter("wvp", [P, 4, D], MMDT, isOutput=False)
    woT2 = nc.declare_dram_parameter("woT2", [P, 4, D], MMDT, isOutput=False)
    bq = nc.declare_dram_parameter("bq", [P, 4], f32, isOutput=False)
    bo_bc = nc.declare_dram_parameter("bo_bc", [P, D], f32, isOutput=False)
    maskp = nc.declare_dram_parameter("maskp", [NPANEL, P, 4, 2, QP], MMDT, isOutput=False)
    ones64 = nc.declare_dram_parameter("ones64", [1, DH], MMDT, isOutput=False)
    out = nc.declare_dram_parameter("out", [NQ, D], f32, isOutput=True)

    with nc.allow_low_precision(reason="bf16 matmul operands"), TileContext(nc) as tc:
        with (
            tc.tile_pool(name="big", bufs=1) as bpool,
            tc.tile_pool(name="attp", bufs=2) as apool,
            tc.tile_pool(name="work", bufs=4) as wpool,
            tc.tile_pool(name="osb", bufs=2) as opool,
            tc.tile_pool(name="ps_proj", bufs=2, space="PSUM") as ps_proj,
            tc.tile_pool(name="ps_s", bufs=2, space="PSUM") as ps_s,
            tc.tile_pool(name="ps_ot", bufs=2, space="PSUM") as ps_ot,
        ):
            # ---- persistent SBUF tensors ----
            xT_sb = bpool.tile([P, 4, 4, 512], MMDT, tag="xT")
            xqT_sb = bpool.tile([P, 2, 4, 512], MMDT, tag="xqT")
            # K^T/Q^T stored head-major on partitions 0-63: matmuls with
            # operands at SBUF base-partition 64 (auto row tile_position)
            # are device-fatal (NRT_EXEC_UNIT_UNRECOVERABLE), so every
            # attention matmul must run at partition base 0.
            kT_sb = bpool.tile([DH, H, S], MMDT, tag="kT")
            v_sb = bpool.tile([P, S // P, H, DH + 1], MMDT, tag="v")
            qT_sb = bpool.tile([DH, H, NQ], MMDT, tag="qT")
            w_sb = {}
            for name in ("wk", "wq", "wv"):
                w_sb[name] = bpool.tile([P, 4, D], MMDT, tag=name, name=name)
            wo_sb = bpool.tile([P, 4, D], MMDT, tag="wo")
            bq_sb = bpool.tile([P, 4], f32, tag="bq")
            bo_sb = bpool.tile([P, D], f32, tag="bo")
            mask_sb = bpool.tile([P, NPANEL, 4, 2, QP], MMDT, tag="mask")
            o64_sb = bpool.tile([1, DH], MMDT, tag="o64")

            # input DMAs in consumption order; every transfer is contiguous
            nc.sync.dma_start(out=w_sb["wk"][:], in_=wkp[:])
            nc.sync.dma_start(out=xT_sb[:, 0], in_=xTp[0])
            nc.sync.dma_start(out=xqT_sb[:, 0], in_=xqTp[0])
            nc.sync.dma_start(out=w_sb["wq"][:], in_=wqp[:])
            nc.sync.dma_start(out=bq_sb[:], in_=bq[:])
            nc.sync.dma_start(out=w_sb["wv"][:], in_=wvp[:])
            nc.sync.dma_start(out=mask_sb[:, 0], in_=maskp[0])
            nc.sync.dma_start(out=o64_sb[:], in_=ones64[:])
            nc.sync.dma_start(out=xT_sb[:, 1], in_=xTp[1])
            nc.sync.dma_start(out=wo_sb[:], in_=woT2[:])
            nc.sync.dma_start(out=bo_sb[:], in_=bo_bc[:])
            nc.sync.dma_start(out=mask_sb[:, 1], in_=maskp[1])
            nc.sync.dma_start(out=xT_sb[:, 2], in_=xTp[2])
            nc.sync.dma_start(out=xqT_sb[:, 1], in_=xqTp[1])
            nc.sync.dma_start(out=mask_sb[:, 2], in_=maskp[2])
            nc.sync.dma_start(out=xT_sb[:, 3], in_=xTp[3])
            nc.sync.dma_start(out=mask_sb[:, 3], in_=maskp[3])
            # ones column appended per head so P.V also yields the softmax
            # denominator in psum row DH for free
            nc.vector.memset(v_sb[:, :, :, DH : DH + 1], 1.0)

            deferred = {"norm": None, "oproj": None}

            def proj_chunk_gen(kc):
                """Yields after each matmul group so the caller can
                interleave projection work into the attention stream."""
                # K^T[j, k-chunk]; no bias (cancels in softmax); head-major
                # split via DVE copies
                for jb in range(4):
                    ps = ps_proj.tile([P, 512], f32, tag="p512")
                    for db in range(4):
                        nc.tensor.matmul(
                            ps[:],
                            lhsT=w_sb["wk"][:, db, jb * P : (jb + 1) * P],
                            rhs=xT_sb[:, kc, db, :],
                            start=(db == 0),
                            stop=(db == 3),
                        )
                    for hh in range(2):
                        nc.vector.tensor_copy(
                            out=kT_sb[:, 2 * jb + hh, kc * 512 : (kc + 1) * 512],
                            in_=ps[hh * DH : (hh + 1) * DH, :],
                        )
                    yield
                # Q^T for a 512-row half (two panels) on even chunks
                if kc % 2 == 0:
                    hf = kc // 2
                    for jb in range(4):
                        ps = ps_proj.tile([P, 512], f32, tag="p512")
                        for db in range(4):
                            nc.tensor.matmul(
                                ps[:],
                                lhsT=w_sb["wq"][:, db, jb * P : (jb + 1) * P],
                                rhs=xqT_sb[:, hf, db, :],
                                start=(db == 0),
                                stop=(db == 3),
                            )
                        for hh in range(2):
                            nc.vector.tensor_tensor(
                                qT_sb[:, 2 * jb + hh, hf * 512 : (hf + 1) * 512],
                                ps[hh * DH : (hh + 1) * DH, :],
                                bq_sb[
                                    hh * DH : (hh + 1) * DH, jb : jb + 1
                                ].to_broadcast([DH, 512]),
                                add_op,
                            )
                        yield
                # V[k-chunk, j]; no bias (folded into bo' host-side)
                for kb in range(4):
                    ps = ps_proj.tile([P, 512], f32, tag="p512")
                    for db in range(4):
                        nc.tensor.matmul(
                            ps[:],
                            lhsT=xT_sb[:, kc, db, kb * P : (kb + 1) * P],
                            rhs=w_sb["wv"][:, db, :],
                            start=(db == 0),
                            stop=(db == 3),
                        )
                    nc.vector.tensor_copy(
                        out=v_sb[:, 4 * kc + kb, :, 0:DH],
                        in_=ps[:].rearrange("p (h d) -> p h d", h=H),
                    )
                    yield

            def make_norm(hp, ot_ps, attT_sb):
                def emit_norm():
                    # attT[:, hp pair, :] = ot / den; den sits in psum row DH.
                    # custom-DVE recip must read SBUF (PSUM input returns
                    # garbage on HW even though CoreSim accepts it).
                    den_sb = wpool.tile([1, 2, QP], f32, tag="den_sb")
                    nc.vector.tensor_copy(out=den_sb[:], in_=ot_ps[DH : DH + 1, :, :])
                    rden_f = wpool.tile([1, 2, QP], f32, tag="rden_f")
                    nc.vector.reciprocal_approx_fast(out=rden_f[:], in_=den_sb[:])
                    rden = wpool.tile([1, 2, QP], MMDT, tag="rden")
                    nc.vector.tensor_copy(out=rden[:], in_=rden_f[:])
                    bc_full = ps_s.tile([P, 2, 2, QP], f32, tag="s", name="bc")
                    bc_ps = bc_full[0:DH, 0, :, :]
                    nc.tensor.matmul(
                        bc_ps, lhsT=o64_sb[:], rhs=rden[:], start=True, stop=True
                    )
                    # DVE cannot read two PSUM operands in one op, so stage
                    # the broadcast through SBUF
                    bc_sb = wpool.tile([DH, 2, QP], f32, tag="bc_sb")
                    nc.vector.tensor_copy(out=bc_sb[:], in_=bc_ps)
                    for hh in range(2):
                        nc.vector.tensor_mul(
                            out=attT_sb[hh * DH : (hh + 1) * DH, hp, :],
                            in0=ot_ps[0:DH, hh, :],
                            in1=bc_sb[:, hh, :],
                        )

                return emit_norm

            def make_oproj(p, attT_sb):
                def emit_oproj():
                    # out[q,:] = attT.T @ Wo.T + bo'; head pairs packed so the
                    # contraction uses all 128 partitions
                    for qs in range(2):
                        ps = ps_proj.tile([P, D], f32, tag="p512")
                        for hp in range(4):
                            nc.tensor.matmul(
                                ps[:],
                                lhsT=attT_sb[:, hp, qs * P : (qs + 1) * P],
                                rhs=wo_sb[:, hp, :],
                                start=(hp == 0),
                                stop=(hp == 3),
                            )
                        osb = opool.tile([P, D], f32, tag="osb")
                        nc.vector.tensor_tensor(osb[:], ps[:], bo_sb[:], add_op)
                        nc.sync.dma_start(
                            out=out[p * QP + qs * P : p * QP + (qs + 1) * P, :],
                            in_=osb[:],
                        )

                return emit_oproj

            def emit_attention_panel(p, gen):
                nblk = 4 * (p + 1)
                nbat = nblk // 2  # 2 k-blocks per exp batch
                q0 = p * QP
                norm_at = 1 if nbat == 2 else 2
                attT_sb = apool.tile([P, 4, QP], MMDT, tag="attT")
                for hp in range(4):  # head pairs (2hp, 2hp+1)
                    ot_ps = ps_ot.tile([DH + 1, 2, QP], f32, tag="ot")

                    def emit_pv(bb, pT, hp=hp, nbat=nbat, ot_ps=ot_ps):
                        for kbi in range(2):
                            for hh in range(2):
                                h = 2 * hp + hh
                                nc.tensor.matmul(
                                    ot_ps[:, hh, :],
                                    lhsT=v_sb[:, 2 * bb + kbi, h, :],
                                    rhs=pT[:, kbi, hh, :],
                                    start=(bb == 0 and kbi == 0 and hh == 0),
                                    stop=(bb == nbat - 1 and kbi == 1 and hh == 1),
                                )

                    # software pipeline: PV for batch bb-1 is emitted after
                    # scores+exp of batch bb, so the in-order PE queue always
                    # has ready matmuls while ACT runs the exp
                    pending = []
                    for bb in range(nbat):
                        s_ps = ps_s.tile([P, 2, 2, QP], f32, tag="s")
                        for kbi in range(2):
                            kb = 2 * bb + kbi
                            for hh in range(2):
                                h = 2 * hp + hh
                                nc.tensor.matmul(
                                    s_ps[:, kbi, hh, :],
                                    lhsT=kT_sb[:, h, kb * P : (kb + 1) * P],
                                    rhs=qT_sb[:, h, q0 : q0 + QP],
                                    start=True,
                                    stop=True,
                                )
                        pT = wpool.tile([P, 2, 2, QP], MMDT, tag="pT")
                        nc.scalar.activation(pT[:], s_ps[:], Exp, scale=SCALE)
                        for kbi in range(2):
                            kb = 2 * bb + kbi
                            if kb >= nblk - 4:
                                # zero masked probabilities; mask pre-expanded
                                # over the head dim -> no broadcast operand ->
                                # DVE 2x mode
                                i = kb - (nblk - 4)
                                nc.vector.tensor_tensor(
                                    pT[:, kbi, :, :],
                                    pT[:, kbi, :, :],
                                    mask_sb[:, p, i, :, :],
                                    mult_op,
                                )
                        pending.append((bb, pT))
                        if bb == norm_at and deferred["norm"] is not None:
                            deferred["norm"]()
                            deferred["norm"] = None
                        if hp == 0 and bb == nbat - 1 and deferred["oproj"] is not None:
                            deferred["oproj"]()
                            deferred["oproj"] = None
                        if len(pending) > 2:
                            emit_pv(*pending.pop(0))
                        # keep PE dense: pull next projection group for the
                        # following k-chunk while ACT digests this batch
                        if gen is not None:
                            next(gen, None)
                    for item in pending:
                        emit_pv(*item)
                    deferred["norm"] = make_norm(hp, ot_ps, attT_sb)
                # drain any leftover projection groups of the next chunk
                if gen is not None:
                    for _ in gen:
                        pass
                deferred["oproj"] = make_oproj(p, attT_sb)

            for _ in proj_chunk_gen(0):
                pass
            for p in range(NPANEL):
                gen = proj_chunk_gen(p + 1) if p < NPANEL - 1 else None
                emit_attention_panel(p, gen)
            deferred["norm"]()
            deferred["oproj"]()
    return nc


_NC = None


def _get_nc():
    global _NC
    if _NC is None:
        _NC = build()
        # run_bass_via_pjrt does not finalize; Bacc.finalize runs the compile
        # passes (register allocation, event-semaphore wait splitting).
        _NC.finalize()
    return _NC


def _qrows(half):
    return np.concatenate(
        [np.arange(512 * p + 256 * half, 512 * p + 256 * half + QP) for p in range(NPANEL)]
    )


def _mask_for(half, mmnp):
    m = np.empty((NPANEL, P, 4, 2, QP), np.float32)
    r = np.arange(P)[:, None]
    c = np.arange(QP)[None, :]
    for p in range(NPANEL):
        q0 = 512 * p + 256 * half
        for i in range(4):
            k = (4 * p + i) * P + r
            mi = np.where(k <= q0 + c, 1.0, 0.0)
            m[p, :, i, 0, :] = mi
            m[p, :, i, 1, :] = mi
    return np.ascontiguousarray(m).astype(mmnp)


def _pack_w(w):
    # [p, db, j] with contraction row d = db*128 + p
    return np.ascontiguousarray(w.T.reshape(4, P, D).transpose(1, 0, 2))


def _in_maps(inputs):
    mmnp = mybir.dt.np(MMDT)
    x = np.asarray(inputs["x"], np.float32)
    wq = _pack_w(np.asarray(inputs["W_Q_w"], np.float32)).astype(mmnp)
    wk = _pack_w(np.asarray(inputs["W_K_w"], np.float32)).astype(mmnp)
    wv = _pack_w(np.asarray(inputs["W_V_w"], np.float32)).astype(mmnp)
    woT = np.asarray(inputs["W_O_w"], np.float32).T  # [ (h,dh), n ]
    wo2 = np.ascontiguousarray(woT.reshape(4, P, D).transpose(1, 0, 2)).astype(mmnp)
    bq = np.ascontiguousarray(np.asarray(inputs["W_Q_b"], np.float32).reshape(4, P).T)
    # V bias folded through the O projection: bo' = bo + bv @ Wo.T
    bo_eff = np.asarray(inputs["W_O_b"], np.float32) + (
        np.asarray(inputs["W_V_b"], np.float32) @ woT
    )
    bo_bc = np.ascontiguousarray(np.broadcast_to(bo_eff, (P, D)))
    ones64 = np.ones((1, DH), mmnp)
    masks = [_mask_for(0, mmnp), _mask_for(1, mmnp)]
    in_maps = []
    for core in range(8):
        b, half = core // 2, core % 2
        xb = x[b]
        xT = xb.T  # [D, S]
        xTp = np.ascontiguousarray(
            xT.reshape(4, P, 4, 512).transpose(2, 1, 0, 3)
        ).astype(mmnp)
        xqT = xb[_qrows(half)].T  # [D, NQ]
        xqTp = np.ascontiguousarray(
            xqT.reshape(4, P, 2, 512).transpose(2, 1, 0, 3)
        ).astype(mmnp)
        in_maps.append(
            {
                "xTp": xTp,
                "xqTp": xqTp,
                "wqp": wq,
                "wkp": wk,
                "wvp": wv,
                "woT2": wo2,
                "bq": bq,
                "bo_bc": bo_bc,
                "maskp": masks[half],
                "ones64": ones64,
            }
        )
    return in_maps


def _assemble(results, B=4):
    out = np.empty((B, S, D), np.float32)
    for core in range(8):
        b, half = core // 2, core % 2
        out[b, _qrows(half), :] = results[core]["out"]
    return out


def run(inputs, trace=False, **kw):
    from concourse.bass_utils import run_bass_kernel_spmd

    res = run_bass_kernel_spmd(
        _get_nc(), _in_maps(inputs), core_ids=list(range(8)), trace=trace, **kw
    )
    return _assemble(res.results), res


def kernel(**inputs):
    out, _ = run(inputs, trace=False)
    return out
